# revision 22
# baseline (speedup 1.0000x reference)
"""3-layer GRU encoder (B=64, S=512, H=512, EMB=300) on 8 Trainium2 cores.

Strategy: data-parallel over batch (8 sequences per core). Per core,
everything runs in a transposed layout (feature dim on SBUF partitions,
tokens on the free dim, token order (t, b)):

  - embedding gather via dma_gather(transpose=True) from an fp16 table
    padded to 384 columns -> x0T [128, 3k, tokens]
  - input projections as chunked matmuls (Wih tiles stationary, tokens
    streaming), bias fused into the PSUM->SBUF move on ScalarE
  - the sequential scan: per step 48 fp16 matmuls (WhhT tiles stationary,
    h.T [128,8] streaming), gates accumulate in PSUM [128, 12 m-tiles, 8],
    gate nonlinearities per 128-row H-chunk, h kept in a 4-tile rotation
    so the gate tail of step t hides under the matmuls of step t+1.
"""

import numpy as np

B, S, H, EMB, VOCAB, L = 64, 512, 512, 300, 32000, 3
NCORES = 8
BL = B // NCORES            # sequences per core
KE = 384                    # padded embedding dim (3 k-chunks of 128)
G3 = 3 * H                  # 1536 gate rows -> 12 m-tiles
NM = G3 // 128              # 12
CH = 64                     # steps per chunk
NCH = S // CH               # chunks
TOK = BL * S                # tokens per core, token i = t*BL + b
CTOK = BL * CH              # tokens per chunk (512)

_cache = {}


def _build_program(unroll=8, nch=NCH, layers=L, gates=True, do_scan=True,
                   scan_repeat=1):
    import concourse.bass as bass
    import concourse.tile as tile
    from concourse import bacc, mybir
    from concourse.bass import ds
    from concourse.masks import make_identity

    f16 = mybir.dt.float16
    f32 = mybir.dt.float32
    i16 = mybir.dt.int16
    ACT = mybir.ActivationFunctionType
    ALU = mybir.AluOpType

    nc = bacc.Bacc(None, target_bir_lowering=False, debug=False)

    # ---- DRAM I/O ----
    idxw = nc.dram_tensor("idxw", [128, TOK // 16], i16, kind="ExternalInput")
    emb = nc.dram_tensor("emb", [VOCAB, KE], f16, kind="ExternalInput")
    wih = [
        nc.dram_tensor(f"wih{l}", [128, 3 if l == 0 else 4, NM, 128], f16,
                       kind="ExternalInput")
        for l in range(L)
    ]
    whh = [
        nc.dram_tensor(f"whh{l}", [128, 4, NM, 128], f16, kind="ExternalInput")
        for l in range(L)
    ]
    bxg = [
        nc.dram_tensor(f"bxg{l}", [128, NM], f32, kind="ExternalInput")
        for l in range(L)
    ]
    bhn = [
        nc.dram_tensor(f"bhn{l}", [1, 4, 128], f16, kind="ExternalInput")
        for l in range(L)
    ]
    h0t = nc.dram_tensor("h0t", [128, L, 4, BL], f16, kind="ExternalInput")
    out2 = nc.dram_tensor("out2", [BL, S, H], f32, kind="ExternalOutput")
    hfin = nc.dram_tensor("hfin", [L, BL, H], f32, kind="ExternalOutput")

    # out2 viewed [chunk, slice(128 tok), t, b, h] for the untranspose DMA
    out2_v = out2.rearrange("b (c s t) h -> c s t b h", c=NCH, s=CTOK // 128, t=128 // BL)

    with tile.TileContext(nc) as tc:
        import contextlib
        with contextlib.ExitStack() as ctx:
            const = ctx.enter_context(tc.tile_pool(name="const", bufs=1))
            xgp = ctx.enter_context(tc.tile_pool(name="xgp", bufs=2))
            otp = ctx.enter_context(tc.tile_pool(name="otp", bufs=2))
            tmp = ctx.enter_context(tc.tile_pool(name="tmp", bufs=4))
            obp = ctx.enter_context(tc.tile_pool(name="obp", bufs=2))
            ps_scan = ctx.enter_context(tc.tile_pool(name="ps_scan", bufs=2, space="PSUM"))
            ps_proj = ctx.enter_context(tc.tile_pool(name="ps_proj", bufs=2, space="PSUM"))
            ps_tr = ctx.enter_context(tc.tile_pool(name="ps_tr", bufs=1, space="PSUM"))

            # ---- load constants to SBUF ----
            idx_sb = const.tile([128, TOK // 16], i16)
            nc.sync.dma_start(idx_sb[:], idxw[:])
            wih_sb, whh_sb, bxg_sb, bhn_sb = [], [], [], []
            for l in range(L):
                w1 = const.tile([128, 3 if l == 0 else 4, NM, 128], f16, tag=f"wih{l}")
                nc.sync.dma_start(w1[:], wih[l][:])
                wih_sb.append(w1)
                w2 = const.tile([128, 4, NM, 128], f16, tag=f"whh{l}")
                nc.sync.dma_start(w2[:], whh[l][:])
                whh_sb.append(w2)
                b1 = const.tile([128, NM], f32, tag=f"bxg{l}")
                nc.sync.dma_start(b1[:], bxg[l][:])
                bxg_sb.append(b1)
                b2 = const.tile([1, 4, 128], f16, tag=f"bhn{l}")
                nc.sync.dma_start(b2[:], bhn[l][:])
                bhn_sb.append(b2)
            h0t_sb = const.tile([128, L, 4, BL], f16)
            nc.sync.dma_start(h0t_sb[:], h0t[:])
            ident = const.tile([128, 128], f16)
            make_identity(nc, ident)
            ones = const.tile([1, BL], f16)
            nc.vector.memset(ones[:], 1.0)

            # ---- embedding gather (transposed): x0T[p, c, k, i] = emb[idx, k*128+p]
            x0T = const.tile([128, NCH, 3, CTOK], f16)
            for c in range(NCH):
                nc.gpsimd.dma_gather(
                    x0T[:, c, :, :],
                    emb[:],
                    idx_sb[:, c * (CTOK // 16):(c + 1) * (CTOK // 16)],
                    CTOK,
                    CTOK,
                    KE,
                    transpose=True,
                )

            # persistent h tiles (4-rotation) per layer
            h_tiles = [
                [const.tile([128, 4, BL], f16, name=f"h{l}_{i}", tag=f"h{l}_{i}")
                 for i in range(unroll)]
                for l in range(L)
            ]

            def proj_chunk(l, c, xg_sb, in_chunk, nk):
                # xg_sb[:, m, :] = Wih_l.T-tile-m @ in_chunk + bias
                for m in range(NM):
                    pp = ps_proj.tile([128, CTOK], f32)
                    for k in range(nk):
                        nc.tensor.matmul(
                            pp[:],
                            wih_sb[l][:, k, m, :],
                            in_chunk[:, k, :],
                            start=(k == 0),
                            stop=(k == nk - 1),
                        )
                    nc.scalar.activation(
                        xg_sb[:, m, :], pp[:], ACT.Identity,
                        bias=bxg_sb[l][:, m:m + 1],
                    )

            def scan_chunk(l, xg_sb, outT):
                hts = h_tiles[l]

                xg_v = xg_sb.rearrange("p (g j) b -> p g j b", g=3)

                def step(u, tok0):
                    # u: unroll position (python int), tok0: RV token offset
                    h_prev = hts[u % unroll]
                    h_new = hts[(u + 1) % unroll]
                    for jj in range(2):  # H-chunk pairs, own psum bank each
                        j0 = jj * 2
                        ps = ps_scan.tile([128, 3, 2, BL], f32,
                                          name=f"psj{jj}", tag=f"psj{jj}")
                        for g in range(3):
                            for dj in range(2):
                                m = g * 4 + j0 + dj
                                for k in range(4):
                                    nc.tensor.matmul(
                                        ps[:, g, dj, :],
                                        whh_sb[l][:, k, m, :],
                                        h_prev[:, k, :],
                                        start=(k == 0),
                                        stop=(k == 3 and g != 2),
                                    )
                                if g == 2:
                                    nc.tensor.matmul(
                                        ps[:, g, dj, :],
                                        bhn_sb[l][:, j0 + dj, :], ones[:],
                                        start=False, stop=True)
                        if not gates:
                            continue
                        grz = tmp.tile([128, 2, 2, BL], f32, tag="grz")
                        nc.vector.tensor_add(
                            grz[:], ps[:, 0:2, :, :],
                            xg_v[:, 0:2, j0:j0 + 2, ds(tok0, BL)])
                        srz = tmp.tile([128, 2, 2, BL], f16, tag="srz")
                        nc.scalar.activation(srz[:], grz[:], ACT.Sigmoid)
                        npre = tmp.tile([128, 2, BL], f32, tag="npre")
                        nc.vector.tensor_mul(npre[:], ps[:, 2, :, :],
                                             srz[:, 0, :, :])
                        npre2 = tmp.tile([128, 2, BL], f32, tag="npre2")
                        nc.vector.tensor_add(
                            npre2[:], npre[:],
                            xg_v[:, 2, j0:j0 + 2, ds(tok0, BL)])
                        n_t = tmp.tile([128, 2, BL], f16, tag="n_t")
                        nc.scalar.activation(n_t[:], npre2[:], ACT.Tanh)
                        d = tmp.tile([128, 2, BL], f16, tag="d")
                        nc.vector.tensor_sub(d[:], h_prev[:, j0:j0 + 2, :], n_t[:])
                        dz = tmp.tile([128, 2, BL], f16, tag="dz")
                        nc.vector.tensor_mul(dz[:], d[:], srz[:, 1, :, :])
                        nc.vector.tensor_add(h_new[:, j0:j0 + 2, :], dz[:], n_t[:])
                    # record h_new into the transposed output buffer
                    if gates:
                        nc.vector.tensor_copy(outT[:, :, ds(tok0, BL)], h_new[:])

                un = [0]

                def body(tok0):
                    step(un[0], tok0)
                    un[0] += 1

                tc.For_i_unrolled(0, CH * BL, BL, body, max_unroll=unroll)
                if not gates:
                    nc.vector.memset(outT[:], 0.0)

            def untranspose_out(c, outT):
                for s in range(CTOK // 128):
                    pt = ps_tr.tile([128, 4, 128], f16)
                    for j in range(4):
                        nc.tensor.transpose(
                            pt[:, j, :], outT[:, j, ds(s * 128, 128)], ident[:])
                    ob = obp.tile([128, 4, 128], f32, tag="ob")
                    nc.vector.tensor_copy(ob[:], pt[:])
                    nc.sync.dma_start(out2_v[c, s], ob[:])

            def write_hfin(l):
                pt = ps_tr.tile([BL, 4, 128], f16, tag="pth")
                for j in range(4):
                    nc.tensor.transpose(
                        pt[:, j, :], h_tiles[l][0][:, j, :], ident[:])
                hb = obp.tile([BL, 4, 128], f32, tag="hb")
                nc.vector.tensor_copy(hb[:], pt[:])
                nc.sync.dma_start(hfin[l], hb[:])

            # init h from h0t
            for l in range(L):
                nc.vector.tensor_copy(h_tiles[l][0][:], h0t_sb[:, l])

            for c in range(nch):
                oT_prev = None
                for l in range(layers):
                    xg = xgp.tile([128, NM, CTOK], f32, tag="xg", name=f"xg_{l}_{c}")
                    proj_chunk(l, c, xg,
                               x0T[:, c] if l == 0 else oT_prev,
                               3 if l == 0 else 4)
                    oT = otp.tile([128, 4, CTOK], f16, tag=f"oT{l}",
                                  name=f"oT{l}_{c}")
                    if do_scan:
                        for _rep in range(scan_repeat):
                            scan_chunk(l, xg, oT)
                    else:
                        nc.vector.tensor_copy(oT[:, 0, 0:BL], h_tiles[l][0][:, 0, :])
                    oT_prev = oT
                untranspose_out(c, oT_prev)
            for l in range(layers):
                write_hfin(l)

    nc.finalize()
    return nc


def _prep_core_inputs(core, input_tensor, hidden, emb16, weights):
    """Per-core input map. weights = dict of prepped shared arrays."""
    b0 = core * BL
    idx = np.asarray(input_tensor[b0:b0 + BL], dtype=np.int64)  # [BL, S]
    flat = idx.T.reshape(-1)                                    # (t, b) order
    w16 = flat.reshape(TOK // 16, 16).T.astype(np.int16)        # [16, TOK//16]
    idxw = np.tile(w16, (8, 1))                                 # [128, TOK//16]

    hs = np.asarray(hidden[:, b0:b0 + BL, :], dtype=np.float32)  # [L, BL, H]
    # h0t[p, l, k, b] = hidden[l, b, k*128+p]
    h0t = hs.transpose(0, 2, 1).reshape(L, 4, 128, BL).transpose(2, 0, 1, 3)
    h0t = np.ascontiguousarray(h0t, dtype=np.float16)

    m = {"idxw": np.ascontiguousarray(idxw), "emb": emb16, "h0t": h0t}
    m.update(weights)
    return m


def _prep_weights(kw):
    """Shared (replicated) weight arrays in device layout."""
    out = {}
    for l in range(L):
        Wih = np.asarray(kw[f"Wih{l}"], dtype=np.float32)   # [1536, in]
        Whh = np.asarray(kw[f"Whh{l}"], dtype=np.float32)   # [1536, 512]
        bih = np.asarray(kw[f"bih{l}"], dtype=np.float32)
        bhh = np.asarray(kw[f"bhh{l}"], dtype=np.float32)
        kin = Wih.shape[1]
        nk = 3 if l == 0 else 4
        kpad = nk * 128
        WihT = np.zeros((kpad, G3), np.float32)
        WihT[:kin] = Wih.T
        out[f"wih{l}"] = np.ascontiguousarray(
            WihT.reshape(nk, 128, NM, 128).transpose(1, 0, 2, 3), dtype=np.float16)
        WhhT = Whh.T  # [512, 1536]
        out[f"whh{l}"] = np.ascontiguousarray(
            WhhT.reshape(4, 128, NM, 128).transpose(1, 0, 2, 3), dtype=np.float16)
        bx = bih + np.concatenate([bhh[:H], bhh[H:2 * H], np.zeros(H, np.float32)])
        out[f"bxg{l}"] = np.ascontiguousarray(bx.reshape(NM, 128).T)
        out[f"bhn{l}"] = np.ascontiguousarray(
            bhh[2 * H:].reshape(1, 4, 128), dtype=np.float16)
    return out


def kernel(input_tensor, hidden, emb_table, **kw):
    from concourse.bass_utils import run_bass_kernel_spmd

    if "nc" not in _cache:
        _cache["nc"] = _build_program()
    nc = _cache["nc"]

    emb16 = np.zeros((VOCAB, KE), np.float16)
    emb16[:, :EMB] = np.asarray(emb_table, dtype=np.float32)
    weights = _prep_weights(kw)

    in_maps = [
        _prep_core_inputs(c, input_tensor, np.asarray(hidden), emb16, weights)
        for c in range(NCORES)
    ]
    _cache["in_maps"] = in_maps
    res = run_bass_kernel_spmd(nc, in_maps, core_ids=list(range(NCORES)))
    _cache["last_result"] = res

    out = np.concatenate([r["out2"] for r in res.results], axis=0)  # [B, S, H]
    hT = np.concatenate([r["hfin"] for r in res.results], axis=1)   # [L, B, H]
    return out.astype(np.float32), hT.astype(np.float32)


# revision 28
# speedup vs baseline: 1.3061x; 1.3061x over previous
"""3-layer GRU encoder (B=64, S=512, H=512, EMB=300) on 8 Trainium2 cores.

Strategy: data-parallel over batch (8 sequences per core). Per core,
everything runs in a transposed layout (feature dim on SBUF partitions,
tokens on the free dim, token order (t, b)):

  - embedding gather via dma_gather(transpose=True) from an fp16 table
    padded to 384 columns -> x0T [128, 3k, tokens]
  - input projections as chunked matmuls (Wih tiles stationary, tokens
    streaming), bias fused into the PSUM->SBUF move on ScalarE
  - the sequential scan: per step 48 fp16 matmuls (WhhT tiles stationary,
    h.T [128,8] streaming), gates accumulate in PSUM split into two banks
    by H-chunk pair (avoids bank-overlap serialization of gate reads vs
    matmul writes); the n-gate bias folds in as a rank-1 K=1 matmul; h
    lives in an unroll-sized rotating tile ring.
  - the three layers' scans run as a wavefront (layer l on chunk sc-l),
    with all active layers' steps interleaved inside one hardware loop:
    each layer's gate-latency tail hides behind the other layers'
    matmuls. Measured ~3.0us per layer-step (weight-flow floor is
    ~2.6us: Whh must stream through the PE array every step).
"""

import numpy as np

B, S, H, EMB, VOCAB, L = 64, 512, 512, 300, 32000, 3
NCORES = 8
BL = B // NCORES            # sequences per core
KE = 384                    # padded embedding dim (3 k-chunks of 128)
G3 = 3 * H                  # 1536 gate rows -> 12 m-tiles
NM = G3 // 128              # 12
CH = 32                     # steps per chunk
NCH = S // CH               # chunks
TOK = BL * S                # tokens per core, token i = t*BL + b
CTOK = BL * CH              # tokens per chunk (512)

_cache = {}


def _build_program(unroll=8, nch=NCH, layers=L, gates=True, do_scan=True,
                   scan_repeat=1):
    import concourse.bass as bass
    import concourse.tile as tile
    from concourse import bacc, mybir
    from concourse.bass import ds
    from concourse.masks import make_identity

    f16 = mybir.dt.float16
    f32 = mybir.dt.float32
    i16 = mybir.dt.int16
    ACT = mybir.ActivationFunctionType
    ALU = mybir.AluOpType

    nc = bacc.Bacc(None, target_bir_lowering=False, debug=False)

    # ---- DRAM I/O ----
    idxw = nc.dram_tensor("idxw", [128, TOK // 16], i16, kind="ExternalInput")
    emb = nc.dram_tensor("emb", [VOCAB, KE], f16, kind="ExternalInput")
    wih = [
        nc.dram_tensor(f"wih{l}", [128, 3 if l == 0 else 4, NM, 128], f16,
                       kind="ExternalInput")
        for l in range(L)
    ]
    whh = [
        nc.dram_tensor(f"whh{l}", [128, 4, NM, 128], f16, kind="ExternalInput")
        for l in range(L)
    ]
    bxg = [
        nc.dram_tensor(f"bxg{l}", [128, NM], f32, kind="ExternalInput")
        for l in range(L)
    ]
    bhn = [
        nc.dram_tensor(f"bhn{l}", [1, 4, 128], f16, kind="ExternalInput")
        for l in range(L)
    ]
    h0t = nc.dram_tensor("h0t", [128, L, 4, BL], f16, kind="ExternalInput")
    out2 = nc.dram_tensor("out2", [BL, S, H], f32, kind="ExternalOutput")
    hfin = nc.dram_tensor("hfin", [L, BL, H], f32, kind="ExternalOutput")

    # out2 viewed [chunk, slice(128 tok), t, b, h] for the untranspose DMA
    out2_v = out2.rearrange("b (c s t) h -> c s t b h", c=NCH, s=CTOK // 128, t=128 // BL)

    with tile.TileContext(nc) as tc:
        import contextlib
        with contextlib.ExitStack() as ctx:
            const = ctx.enter_context(tc.tile_pool(name="const", bufs=1))
            xgp = ctx.enter_context(tc.tile_pool(name="xgp", bufs=1))
            otp = ctx.enter_context(tc.tile_pool(name="otp", bufs=2))
            tmp = ctx.enter_context(tc.tile_pool(name="tmp", bufs=8))
            obp = ctx.enter_context(tc.tile_pool(name="obp", bufs=2))
            ps_scan = ctx.enter_context(tc.tile_pool(name="ps_scan", bufs=2, space="PSUM"))
            ps_proj = ctx.enter_context(tc.tile_pool(name="ps_proj", bufs=2, space="PSUM"))
            ps_tr = ctx.enter_context(tc.tile_pool(name="ps_tr", bufs=1, space="PSUM"))

            # ---- load constants to SBUF ----
            idx_sb = const.tile([128, TOK // 16], i16)
            nc.sync.dma_start(idx_sb[:], idxw[:])
            wih_sb, whh_sb, bxg_sb, bhn_sb = [], [], [], []
            for l in range(L):
                w1 = const.tile([128, 3 if l == 0 else 4, NM, 128], f16, tag=f"wih{l}")
                nc.sync.dma_start(w1[:], wih[l][:])
                wih_sb.append(w1)
                w2 = const.tile([128, 4, NM, 128], f16, tag=f"whh{l}")
                nc.sync.dma_start(w2[:], whh[l][:])
                whh_sb.append(w2)
                b1 = const.tile([128, NM], f32, tag=f"bxg{l}")
                nc.sync.dma_start(b1[:], bxg[l][:])
                bxg_sb.append(b1)
                b2 = const.tile([1, 4, 128], f16, tag=f"bhn{l}")
                nc.sync.dma_start(b2[:], bhn[l][:])
                bhn_sb.append(b2)
            h0t_sb = const.tile([128, L, 4, BL], f16)
            nc.sync.dma_start(h0t_sb[:], h0t[:])
            ident = const.tile([128, 128], f16)
            make_identity(nc, ident)
            ones = const.tile([1, BL], f16)
            nc.vector.memset(ones[:], 1.0)

            # ---- embedding gather (transposed): x0T[p, c, k, i] = emb[idx, k*128+p]
            x0T = const.tile([128, NCH, 3, CTOK], f16)
            for c in range(NCH):
                nc.gpsimd.dma_gather(
                    x0T[:, c, :, :],
                    emb[:],
                    idx_sb[:, c * (CTOK // 16):(c + 1) * (CTOK // 16)],
                    CTOK,
                    CTOK,
                    KE,
                    transpose=True,
                )

            # persistent h tiles (4-rotation) per layer
            h_tiles = [
                [const.tile([128, 4, BL], f16, name=f"h{l}_{i}", tag=f"h{l}_{i}")
                 for i in range(unroll)]
                for l in range(L)
            ]

            def proj_chunk(l, c, xg_sb, in_chunk, nk):
                # xg_sb[:, m, :] = Wih_l.T-tile-m @ in_chunk + bias
                for m in range(NM):
                    pp = ps_proj.tile([128, CTOK], f32)
                    for k in range(nk):
                        nc.tensor.matmul(
                            pp[:],
                            wih_sb[l][:, k, m, :],
                            in_chunk[:, k, :],
                            start=(k == 0),
                            stop=(k == nk - 1),
                        )
                    nc.scalar.activation(
                        xg_sb[:, m, :], pp[:], ACT.Identity,
                        bias=bxg_sb[l][:, m:m + 1],
                    )

            def emit_step(l, xg_v, outT, u, tok0):
                    # u: unroll position (python int), tok0: RV token offset
                    hts = h_tiles[l]
                    h_prev = hts[u % unroll]
                    h_new = hts[(u + 1) % unroll]
                    for jj in range(2):  # H-chunk pairs, own psum bank each
                        j0 = jj * 2
                        ps = ps_scan.tile([128, 3, 2, BL], f32,
                                          name=f"psj{jj}", tag=f"psj{jj}")
                        for g in range(3):
                            for dj in range(2):
                                m = g * 4 + j0 + dj
                                for k in range(4):
                                    nc.tensor.matmul(
                                        ps[:, g, dj, :],
                                        whh_sb[l][:, k, m, :],
                                        h_prev[:, k, :],
                                        start=(k == 0),
                                        stop=(k == 3 and g != 2),
                                    )
                                if g == 2:
                                    nc.tensor.matmul(
                                        ps[:, g, dj, :],
                                        bhn_sb[l][:, j0 + dj, :], ones[:],
                                        start=False, stop=True)
                        if not gates:
                            continue
                        grz = tmp.tile([128, 2, 2, BL], f32, tag="grz")
                        nc.vector.tensor_add(
                            grz[:], ps[:, 0:2, :, :],
                            xg_v[:, 0:2, j0:j0 + 2, ds(tok0, BL)])
                        srz = tmp.tile([128, 2, 2, BL], f16, tag="srz")
                        nc.scalar.activation(srz[:], grz[:], ACT.Sigmoid)
                        npre = tmp.tile([128, 2, BL], f32, tag="npre")
                        nc.vector.tensor_mul(npre[:], ps[:, 2, :, :],
                                             srz[:, 0, :, :])
                        npre2 = tmp.tile([128, 2, BL], f32, tag="npre2")
                        nc.vector.tensor_add(
                            npre2[:], npre[:],
                            xg_v[:, 2, j0:j0 + 2, ds(tok0, BL)])
                        n_t = tmp.tile([128, 2, BL], f16, tag="n_t")
                        nc.scalar.activation(n_t[:], npre2[:], ACT.Tanh)
                        d = tmp.tile([128, 2, BL], f16, tag="d")
                        nc.vector.tensor_sub(d[:], h_prev[:, j0:j0 + 2, :], n_t[:])
                        dz = tmp.tile([128, 2, BL], f16, tag="dz")
                        nc.vector.tensor_mul(dz[:], d[:], srz[:, 1, :, :])
                        nc.vector.tensor_add(h_new[:, j0:j0 + 2, :], dz[:], n_t[:])
                    # record h_new into the transposed output buffer
                    if gates:
                        nc.vector.tensor_copy(outT[:, :, ds(tok0, BL)], h_new[:])

            def untranspose_out(c, outT):
                for s in range(CTOK // 128):
                    pt = ps_tr.tile([128, 4, 128], f16)
                    for j in range(4):
                        nc.tensor.transpose(
                            pt[:, j, :], outT[:, j, ds(s * 128, 128)], ident[:])
                    ob = obp.tile([128, 4, 128], f32, tag="ob")
                    nc.vector.tensor_copy(ob[:], pt[:])
                    nc.sync.dma_start(out2_v[c, s], ob[:])

            def write_hfin(l):
                pt = ps_tr.tile([BL, 4, 128], f16, tag="pth")
                for j in range(4):
                    nc.tensor.transpose(
                        pt[:, j, :], h_tiles[l][0][:, j, :], ident[:])
                hb = obp.tile([BL, 4, 128], f32, tag="hb")
                nc.vector.tensor_copy(hb[:], pt[:])
                nc.sync.dma_start(hfin[l], hb[:])

            # init h from h0t
            for l in range(L):
                nc.vector.tensor_copy(h_tiles[l][0][:], h0t_sb[:, l])

            # Wavefront over layers: superchunk sc runs layer l on chunk
            # (sc - l); the three layers' steps interleave inside one HW
            # loop so each layer's gate-latency tail hides behind the
            # other layers' matmuls.
            prev_oT = {}
            for sc in range(nch + layers - 1):
                active = [l for l in range(layers) if 0 <= sc - l < nch]
                xgv_t, oT_t = {}, {}
                for l in active:
                    c = sc - l
                    xg = xgp.tile([128, NM, CTOK], f32, tag=f"xg{l}",
                                  name=f"xg_{l}_{c}")
                    proj_chunk(l, c, xg,
                               x0T[:, c] if l == 0 else prev_oT[l - 1],
                               3 if l == 0 else 4)
                    xgv_t[l] = xg.rearrange("p (g j) b -> p g j b", g=3)
                    oT_t[l] = otp.tile([128, 4, CTOK], f16, tag=f"oT{l}",
                                       name=f"oT{l}_{c}")
                if do_scan and active:
                    for _rep in range(scan_repeat):
                        un = {l: [0] for l in active}

                        def body(tok0):
                            for l in active:
                                emit_step(l, xgv_t[l], oT_t[l], un[l][0], tok0)
                                un[l][0] += 1

                        tc.For_i_unrolled(0, CH * BL, BL, body,
                                          max_unroll=unroll)
                else:
                    for l in active:
                        nc.vector.tensor_copy(oT_t[l][:, 0, 0:BL],
                                              h_tiles[l][0][:, 0, :])
                if not gates and do_scan:
                    for l in active:
                        nc.vector.memset(oT_t[l][:], 0.0)
                for l in active:
                    prev_oT[l] = oT_t[l]
                if layers - 1 in active:
                    untranspose_out(sc - (layers - 1), oT_t[layers - 1])
                for l in active:
                    if sc - l == nch - 1:
                        write_hfin(l)

    nc.finalize()
    return nc


def _prep_core_inputs(core, input_tensor, hidden, emb16, weights):
    """Per-core input map. weights = dict of prepped shared arrays."""
    b0 = core * BL
    idx = np.asarray(input_tensor[b0:b0 + BL], dtype=np.int64)  # [BL, S]
    flat = idx.T.reshape(-1)                                    # (t, b) order
    w16 = flat.reshape(TOK // 16, 16).T.astype(np.int16)        # [16, TOK//16]
    idxw = np.tile(w16, (8, 1))                                 # [128, TOK//16]

    hs = np.asarray(hidden[:, b0:b0 + BL, :], dtype=np.float32)  # [L, BL, H]
    # h0t[p, l, k, b] = hidden[l, b, k*128+p]
    h0t = hs.transpose(0, 2, 1).reshape(L, 4, 128, BL).transpose(2, 0, 1, 3)
    h0t = np.ascontiguousarray(h0t, dtype=np.float16)

    m = {"idxw": np.ascontiguousarray(idxw), "emb": emb16, "h0t": h0t}
    m.update(weights)
    return m


def _prep_weights(kw):
    """Shared (replicated) weight arrays in device layout."""
    out = {}
    for l in range(L):
        Wih = np.asarray(kw[f"Wih{l}"], dtype=np.float32)   # [1536, in]
        Whh = np.asarray(kw[f"Whh{l}"], dtype=np.float32)   # [1536, 512]
        bih = np.asarray(kw[f"bih{l}"], dtype=np.float32)
        bhh = np.asarray(kw[f"bhh{l}"], dtype=np.float32)
        kin = Wih.shape[1]
        nk = 3 if l == 0 else 4
        kpad = nk * 128
        WihT = np.zeros((kpad, G3), np.float32)
        WihT[:kin] = Wih.T
        out[f"wih{l}"] = np.ascontiguousarray(
            WihT.reshape(nk, 128, NM, 128).transpose(1, 0, 2, 3), dtype=np.float16)
        WhhT = Whh.T  # [512, 1536]
        out[f"whh{l}"] = np.ascontiguousarray(
            WhhT.reshape(4, 128, NM, 128).transpose(1, 0, 2, 3), dtype=np.float16)
        bx = bih + np.concatenate([bhh[:H], bhh[H:2 * H], np.zeros(H, np.float32)])
        out[f"bxg{l}"] = np.ascontiguousarray(bx.reshape(NM, 128).T)
        out[f"bhn{l}"] = np.ascontiguousarray(
            bhh[2 * H:].reshape(1, 4, 128), dtype=np.float16)
    return out


def kernel(input_tensor, hidden, emb_table, **kw):
    from concourse.bass_utils import run_bass_kernel_spmd

    if "nc" not in _cache:
        _cache["nc"] = _build_program()
    nc = _cache["nc"]

    emb16 = np.zeros((VOCAB, KE), np.float16)
    emb16[:, :EMB] = np.asarray(emb_table, dtype=np.float32)
    weights = _prep_weights(kw)

    in_maps = [
        _prep_core_inputs(c, input_tensor, np.asarray(hidden), emb16, weights)
        for c in range(NCORES)
    ]
    _cache["in_maps"] = in_maps
    res = run_bass_kernel_spmd(nc, in_maps, core_ids=list(range(NCORES)))
    _cache["last_result"] = res

    out = np.concatenate([r["out2"] for r in res.results], axis=0)  # [B, S, H]
    hT = np.concatenate([r["hfin"] for r in res.results], axis=1)   # [L, B, H]
    return out.astype(np.float32), hT.astype(np.float32)


# revision 34
# speedup vs baseline: 1.3216x; 1.0119x over previous
"""3-layer GRU encoder (B=64, S=512, H=512, EMB=300) on 8 Trainium2 cores.

Strategy: data-parallel over batch (8 sequences per core). Per core,
everything runs in a transposed layout (feature dim on SBUF partitions,
tokens on the free dim, token order (t, b)):

  - embedding gather via dma_gather(transpose=True) from an fp16 table
    padded to 384 columns -> x0T [128, 3k, tokens]
  - input projections as chunked matmuls (Wih tiles stationary, tokens
    streaming), bias fused into the PSUM->SBUF move on ScalarE
  - the sequential scan: per step 48 fp16 matmuls (WhhT tiles stationary,
    h.T [128,8] streaming), gates accumulate in PSUM split into two banks
    by H-chunk pair (avoids bank-overlap serialization of gate reads vs
    matmul writes); the n-gate bias folds in as a rank-1 K=1 matmul; h
    lives in an unroll-sized rotating tile ring.
  - the three layers' scans run as a wavefront (layer l on chunk sc-l),
    with all active layers' steps interleaved inside one hardware loop:
    each layer's gate-latency tail hides behind the other layers'
    matmuls. Measured ~3.0us per layer-step (weight-flow floor is
    ~2.6us: Whh must stream through the PE array every step).
"""

import numpy as np

B, S, H, EMB, VOCAB, L = 64, 512, 512, 300, 32000, 3
NCORES = 8
BL = B // NCORES            # sequences per core
KE = 384                    # padded embedding dim (3 k-chunks of 128)
G3 = 3 * H                  # 1536 gate rows -> 12 m-tiles
NM = G3 // 128              # 12
CH = 32                     # steps per chunk
NCH = S // CH               # chunks
TOK = BL * S                # tokens per core, token i = t*BL + b
CTOK = BL * CH              # tokens per chunk (512)

_cache = {}


def _build_program(unroll=8, nch=NCH, layers=L, gates=True, do_scan=True,
                   scan_repeat=1):
    import concourse.bass as bass
    import concourse.tile as tile
    from concourse import bacc, mybir
    from concourse.bass import ds
    from concourse.masks import make_identity

    f16 = mybir.dt.float16
    f32 = mybir.dt.float32
    i16 = mybir.dt.int16
    ACT = mybir.ActivationFunctionType
    ALU = mybir.AluOpType

    nc = bacc.Bacc(None, target_bir_lowering=False, debug=False)

    # ---- DRAM I/O ----
    idxw = nc.dram_tensor("idxw", [128, TOK // 16], i16, kind="ExternalInput")
    emb = nc.dram_tensor("emb", [VOCAB, KE], f16, kind="ExternalInput")
    wih = [
        nc.dram_tensor(f"wih{l}", [128, 3 if l == 0 else 4, NM, 128], f16,
                       kind="ExternalInput")
        for l in range(L)
    ]
    whh = [
        nc.dram_tensor(f"whh{l}", [128, 4, NM, 128], f16, kind="ExternalInput")
        for l in range(L)
    ]
    bxg = [
        nc.dram_tensor(f"bxg{l}", [128, NM], f32, kind="ExternalInput")
        for l in range(L)
    ]
    bhn = [
        nc.dram_tensor(f"bhn{l}", [128, 4], f32, kind="ExternalInput")
        for l in range(L)
    ]
    h0t = nc.dram_tensor("h0t", [128, L, 4, BL], f16, kind="ExternalInput")
    out2 = nc.dram_tensor("out2", [BL, S, H], f32, kind="ExternalOutput")
    hfin = nc.dram_tensor("hfin", [L, BL, H], f32, kind="ExternalOutput")

    # out2 viewed [chunk, slice(128 tok), t, b, h] for the untranspose DMA
    out2_v = out2.rearrange("b (c s t) h -> c s t b h", c=NCH, s=CTOK // 128, t=128 // BL)

    with tile.TileContext(nc) as tc:
        import contextlib
        with contextlib.ExitStack() as ctx:
            const = ctx.enter_context(tc.tile_pool(name="const", bufs=1))
            xgp = ctx.enter_context(tc.tile_pool(name="xgp", bufs=1))
            otp = ctx.enter_context(tc.tile_pool(name="otp", bufs=2))
            tmp = ctx.enter_context(tc.tile_pool(name="tmp", bufs=8))
            obp = ctx.enter_context(tc.tile_pool(name="obp", bufs=2))
            ps_scan = ctx.enter_context(tc.tile_pool(name="ps_scan", bufs=2, space="PSUM"))
            ps_proj = ctx.enter_context(tc.tile_pool(name="ps_proj", bufs=2, space="PSUM"))
            ps_tr = ctx.enter_context(tc.tile_pool(name="ps_tr", bufs=1, space="PSUM"))

            # ---- load constants to SBUF ----
            idx_sb = const.tile([128, TOK // 16], i16)
            nc.sync.dma_start(idx_sb[:], idxw[:])
            wih_sb, whh_sb, bxg_sb, bhn_sb = [], [], [], []
            for l in range(L):
                w1 = const.tile([128, 3 if l == 0 else 4, NM, 128], f16, tag=f"wih{l}")
                nc.sync.dma_start(w1[:], wih[l][:])
                wih_sb.append(w1)
                w2 = const.tile([128, 4, NM, 128], f16, tag=f"whh{l}")
                nc.sync.dma_start(w2[:], whh[l][:])
                whh_sb.append(w2)
                b1 = const.tile([128, NM], f32, tag=f"bxg{l}")
                nc.sync.dma_start(b1[:], bxg[l][:])
                bxg_sb.append(b1)
                b2 = const.tile([128, 4], f32, tag=f"bhn{l}")
                nc.sync.dma_start(b2[:], bhn[l][:])
                bhn_sb.append(b2)
            h0t_sb = const.tile([128, L, 4, BL], f16)
            nc.sync.dma_start(h0t_sb[:], h0t[:])
            ident = const.tile([128, 128], f16)
            make_identity(nc, ident)

            # ---- embedding gather (transposed): x0T[p, c, k, i] = emb[idx, k*128+p]
            x0T = const.tile([128, NCH, 3, CTOK], f16)
            for c in range(NCH):
                nc.gpsimd.dma_gather(
                    x0T[:, c, :, :],
                    emb[:],
                    idx_sb[:, c * (CTOK // 16):(c + 1) * (CTOK // 16)],
                    CTOK,
                    CTOK,
                    KE,
                    transpose=True,
                )

            # persistent h tiles (4-rotation) per layer
            h_tiles = [
                [const.tile([128, 4, BL], f16, name=f"h{l}_{i}", tag=f"h{l}_{i}")
                 for i in range(unroll)]
                for l in range(L)
            ]

            def proj_chunk(l, c, xg_sb, in_chunk, nk):
                # xg_sb[:, m, :] = Wih_l.T-tile-m @ in_chunk + bias
                for m in range(NM):
                    pp = ps_proj.tile([128, CTOK], f32)
                    for k in range(nk):
                        nc.tensor.matmul(
                            pp[:],
                            wih_sb[l][:, k, m, :],
                            in_chunk[:, k, :],
                            start=(k == 0),
                            stop=(k == nk - 1),
                        )
                    nc.scalar.activation(
                        xg_sb[:, m, :], pp[:], ACT.Identity,
                        bias=bxg_sb[l][:, m:m + 1],
                    )

            def emit_step(l, xg_v, outT, u, tok0):
                    # u: unroll position (python int), tok0: RV token offset
                    hts = h_tiles[l]
                    h_prev = hts[u % unroll]
                    h_new = hts[(u + 1) % unroll]
                    for jj in range(2):  # H-chunk pairs, own psum bank each
                        j0 = jj * 2
                        ps = ps_scan.tile([128, 3, 2, BL], f32,
                                          name=f"psj{jj}", tag=f"psj{jj}")
                        for g in range(3):
                            for dj in range(2):
                                m = g * 4 + j0 + dj
                                for k in range(4):
                                    nc.tensor.matmul(
                                        ps[:, g, dj, :],
                                        whh_sb[l][:, k, m, :],
                                        h_prev[:, k, :],
                                        start=(k == 0),
                                        stop=(k == 3),
                                    )
                        if not gates:
                            continue
                        grz = tmp.tile([128, 2, 2, BL], f32, tag="grz")
                        nc.vector.tensor_add(
                            grz[:], ps[:, 0:2, :, :],
                            xg_v[:, 0:2, j0:j0 + 2, ds(tok0, BL)])
                        srz = tmp.tile([128, 2, 2, BL], f16, tag="srz")
                        nc.scalar.activation(srz[:], grz[:], ACT.Sigmoid)
                        npre = tmp.tile([128, 2, BL], f32, tag="npre")
                        for dj in range(2):
                            # (hn + bhn) * r with bhn as per-partition scalar
                            nc.vector.scalar_tensor_tensor(
                                npre[:, dj, :], ps[:, 2, dj, :],
                                bhn_sb[l][:, j0 + dj:j0 + dj + 1],
                                srz[:, 0, dj, :],
                                op0=ALU.add, op1=ALU.mult)
                        npre2 = tmp.tile([128, 2, BL], f32, tag="npre2")
                        nc.vector.tensor_add(
                            npre2[:], npre[:],
                            xg_v[:, 2, j0:j0 + 2, ds(tok0, BL)])
                        n_t = tmp.tile([128, 2, BL], f16, tag="n_t")
                        nc.scalar.activation(n_t[:], npre2[:], ACT.Tanh)
                        d = tmp.tile([128, 2, BL], f16, tag="d")
                        nc.vector.tensor_sub(d[:], h_prev[:, j0:j0 + 2, :], n_t[:])
                        dz = tmp.tile([128, 2, BL], f16, tag="dz")
                        nc.vector.tensor_mul(dz[:], d[:], srz[:, 1, :, :])
                        nc.vector.tensor_add(h_new[:, j0:j0 + 2, :], dz[:], n_t[:])
                    # record h_new into the transposed output buffer
                    if gates:
                        nc.vector.tensor_copy(outT[:, :, ds(tok0, BL)], h_new[:])

            def untranspose_out(c, outT):
                for s in range(CTOK // 128):
                    pt = ps_tr.tile([128, 4, 128], f16)
                    for j in range(4):
                        nc.tensor.transpose(
                            pt[:, j, :], outT[:, j, ds(s * 128, 128)], ident[:])
                    ob = obp.tile([128, 4, 128], f32, tag="ob")
                    nc.vector.tensor_copy(ob[:], pt[:])
                    nc.sync.dma_start(out2_v[c, s], ob[:])

            def write_hfin(l):
                pt = ps_tr.tile([BL, 4, 128], f16, tag="pth")
                for j in range(4):
                    nc.tensor.transpose(
                        pt[:, j, :], h_tiles[l][0][:, j, :], ident[:])
                hb = obp.tile([BL, 4, 128], f32, tag="hb")
                nc.vector.tensor_copy(hb[:], pt[:])
                nc.sync.dma_start(hfin[l], hb[:])

            # init h from h0t
            for l in range(L):
                nc.vector.tensor_copy(h_tiles[l][0][:], h0t_sb[:, l])

            # Wavefront over layers: superchunk sc runs layer l on chunk
            # (sc - l); the three layers' steps interleave inside one HW
            # loop so each layer's gate-latency tail hides behind the
            # other layers' matmuls.
            prev_oT = {}
            for sc in range(nch + layers - 1):
                active = [l for l in range(layers) if 0 <= sc - l < nch]
                xgv_t, oT_t = {}, {}
                for l in active:
                    c = sc - l
                    xg = xgp.tile([128, NM, CTOK], f32, tag=f"xg{l}",
                                  name=f"xg_{l}_{c}")
                    proj_chunk(l, c, xg,
                               x0T[:, c] if l == 0 else prev_oT[l - 1],
                               3 if l == 0 else 4)
                    xgv_t[l] = xg.rearrange("p (g j) b -> p g j b", g=3)
                    oT_t[l] = otp.tile([128, 4, CTOK], f16, tag=f"oT{l}",
                                       name=f"oT{l}_{c}")
                if do_scan and active:
                    for _rep in range(scan_repeat):
                        un = {l: [0] for l in active}

                        def body(tok0):
                            for l in active:
                                emit_step(l, xgv_t[l], oT_t[l], un[l][0], tok0)
                                un[l][0] += 1

                        tc.For_i_unrolled(0, CH * BL, BL, body,
                                          max_unroll=unroll)
                else:
                    for l in active:
                        nc.vector.tensor_copy(oT_t[l][:, 0, 0:BL],
                                              h_tiles[l][0][:, 0, :])
                if not gates and do_scan:
                    for l in active:
                        nc.vector.memset(oT_t[l][:], 0.0)
                for l in active:
                    prev_oT[l] = oT_t[l]
                if layers - 1 in active:
                    untranspose_out(sc - (layers - 1), oT_t[layers - 1])
                for l in active:
                    if sc - l == nch - 1:
                        write_hfin(l)

    nc.finalize()
    return nc


def _prep_core_inputs(core, input_tensor, hidden, emb16, weights):
    """Per-core input map. weights = dict of prepped shared arrays."""
    b0 = core * BL
    idx = np.asarray(input_tensor[b0:b0 + BL], dtype=np.int64)  # [BL, S]
    flat = idx.T.reshape(-1)                                    # (t, b) order
    w16 = flat.reshape(TOK // 16, 16).T.astype(np.int16)        # [16, TOK//16]
    idxw = np.tile(w16, (8, 1))                                 # [128, TOK//16]

    hs = np.asarray(hidden[:, b0:b0 + BL, :], dtype=np.float32)  # [L, BL, H]
    # h0t[p, l, k, b] = hidden[l, b, k*128+p]
    h0t = hs.transpose(0, 2, 1).reshape(L, 4, 128, BL).transpose(2, 0, 1, 3)
    h0t = np.ascontiguousarray(h0t, dtype=np.float16)

    m = {"idxw": np.ascontiguousarray(idxw), "emb": emb16, "h0t": h0t}
    m.update(weights)
    return m


def _prep_weights(kw):
    """Shared (replicated) weight arrays in device layout."""
    out = {}
    for l in range(L):
        Wih = np.asarray(kw[f"Wih{l}"], dtype=np.float32)   # [1536, in]
        Whh = np.asarray(kw[f"Whh{l}"], dtype=np.float32)   # [1536, 512]
        bih = np.asarray(kw[f"bih{l}"], dtype=np.float32)
        bhh = np.asarray(kw[f"bhh{l}"], dtype=np.float32)
        kin = Wih.shape[1]
        nk = 3 if l == 0 else 4
        kpad = nk * 128
        WihT = np.zeros((kpad, G3), np.float32)
        WihT[:kin] = Wih.T
        out[f"wih{l}"] = np.ascontiguousarray(
            WihT.reshape(nk, 128, NM, 128).transpose(1, 0, 2, 3), dtype=np.float16)
        WhhT = Whh.T  # [512, 1536]
        out[f"whh{l}"] = np.ascontiguousarray(
            WhhT.reshape(4, 128, NM, 128).transpose(1, 0, 2, 3), dtype=np.float16)
        bx = bih + np.concatenate([bhh[:H], bhh[H:2 * H], np.zeros(H, np.float32)])
        out[f"bxg{l}"] = np.ascontiguousarray(bx.reshape(NM, 128).T)
        out[f"bhn{l}"] = np.ascontiguousarray(bhh[2 * H:].reshape(4, 128).T)
    return out


def kernel(input_tensor, hidden, emb_table, **kw):
    from concourse.bass_utils import run_bass_kernel_spmd

    if "nc" not in _cache:
        _cache["nc"] = _build_program()
    nc = _cache["nc"]

    emb16 = np.zeros((VOCAB, KE), np.float16)
    emb16[:, :EMB] = np.asarray(emb_table, dtype=np.float32)
    weights = _prep_weights(kw)

    in_maps = [
        _prep_core_inputs(c, input_tensor, np.asarray(hidden), emb16, weights)
        for c in range(NCORES)
    ]
    _cache["in_maps"] = in_maps
    res = run_bass_kernel_spmd(nc, in_maps, core_ids=list(range(NCORES)))
    _cache["last_result"] = res

    out = np.concatenate([r["out2"] for r in res.results], axis=0)  # [B, S, H]
    hT = np.concatenate([r["hfin"] for r in res.results], axis=1)   # [L, B, H]
    return out.astype(np.float32), hT.astype(np.float32)
